# revision 13
# baseline (speedup 1.0000x reference)
"""Exponentiated-quadratic (RBF) kernel matrix on 8 Trainium2 NeuronCores.

K[i, j] = sigma * exp(-0.5 * ||x1_i/rho - x2_j/rho||^2)
        with sigma = exp(log_sigma)^2, rho = exp(log_rho)

Strategy
--------
Row-shard x1 across the 8 cores (512 rows each), replicate x2. Each core
computes S = (x1/rho) @ (x2/rho)^T - 0.5*||y_j||^2 on the tensor engine and
finishes with one ScalarE activation per PSUM tile:
K = exp(S + (-0.5*||x_i||^2 + 2*log_sigma)), using ACT's free per-partition
bias (exact fp32 for the x-norms) — so the whole epilogue is a single pass.

Key performance facts driving the design (vs the 42us fp32 baseline):
- The exp chain on ScalarE (8 x 2048-wide ACTIVATEs, ~2.0us each at
  1.2GHz, only engine with exp) is the critical resource. Everything else
  is arranged so those 8 ACTIVATEs run back-to-back: no DMA triggers on
  the scalar queue before the last ACT, PSUM ping-pong at 2048 so the PE
  refills one half while ACT drains the other.
- Output is stored as bf16 (0.2% rel err, budget 2e-2) and widened to fp32
  on the host: halves the dominant HBM store traffic (8.4 -> 4.2 MB).
- Inputs ride both HWDGE queues in criticality order (per-queue FIFO):
  sync: l1 (A0 + Bh0 + bias, feeds passes 1-2 of the first tile) then
  l2a (all of h=1); scalar: l1b (Bl0/Yn0/ones, passes 3-4) then l2b
  (A rows 128..512). The first tile's pass order (Ah.Bh, Al.Bh, Ah.Bl,
  ones.Yn) covers l1b's landing; no SWDGE load (its descriptor ring
  contends with SDMA engine 15 and delays every completion semaphore).
- All four row-blocks share one [128, 16384] bf16 stage tile, so stores
  can merge: [i0,i1] go out as ONE 2MB DMA via a (p, b, j) access
  pattern, keeping the total HWDGE DMA count at 8 (= the number of
  DMAHW sem lanes; a 9th would need a lane-reuse ordering wait and
  walrus here rejects instructions with more than one semaphore wait).
  The last two half-block stores split across the two queues so the tail
  transfer after the final ACTIVATE is a single 512KB store per queue.

Matmul precision: 3-pass bf16 split (Ah.Bh + Al.Bh + Ah.Bl, fp32 PSUM
accumulation, the dropped Al.Bl term is ~2^-18 relative) plus a K=3
ones-weighted pass adding the triple-bf16-split -0.5*||y_j||^2 row.

PE utilisation: K=32 fits a 32-row strip of the 128x128 array, so four
matmul streams run CONCURRENTLY via tile_position=(32s, 0) — column slice
q of each PSUM tile runs in strip q. B is packed on the host so strip s's
rows hold exactly the columns strip s consumes; A is replicated per strip;
the fp32 ACT bias rides along bit-cast as bf16 column pairs.

walrus in this container rejects instructions carrying more than one
semaphore wait, which also shapes: persistent PSUM tiles, a tiny ScalarE
copy that "observes" the l1 DMA before the activations (each ACTIVATE
then carries only its PE wait), ACT->ACT pseudo-deps demoted to nosync,
merged stores demoted to wait only on their LAST producer ACT (scalar
FIFO covers the earlier ones), and a chain of single-wait NOPs on the
sync sequencer that funnels every completion into the framework's
kernel-tail drain.
"""

import numpy as np
import ml_dtypes

import concourse.bass as bass
import concourse.mybir as mybir
import concourse.tile as tile
from concourse.bass_utils import run_bass_kernel_spmd
from concourse.tile import add_dep_helper

N, M, P = 4096, 4096, 32
NCORES = 8
NSHARD = N // NCORES  # 512 rows of x1 per core
IBLK = 128            # output row-block = PSUM partition dim
JBLK = 512            # matmul free dim = one fp32 PSUM bank
PSW = 2048            # PSUM tile width (4 banks) = one exp-activation
NSTRIP = 4            # concurrent PE row strips (K=32 each)
NI = NSHARD // IBLK   # 4 row-blocks
NH = M // PSW         # 2 PSUM tiles per row-block

BF16 = mybir.dt.bfloat16
NPBF16 = ml_dtypes.bfloat16

# l1 (sync, critical): passes 1-2 of tile (i0,h0) + the ACT bias.
AHI_O = 0
ALO_O = 128
BH0_O = 256
XN_O = 768
L1_W = 784
# l1b (scalar, lands concurrently): passes 3-4 of h=0.
BL0_O = 0
YN0_O = 512
ONES_O = 1024
L1B_W = 1152
# l2a (sync, 3rd): the h=1 passes 1-3 operands.
BH1_O = 0
BL1_O = 512
L2A_W = 1024
# l2b (sync, last): Yn for h=1 (needed only by its pass 4) and the A
# slabs for row-blocks 1..3 (needed ~2 ACTs later).
YN1_O = 0
AHI3_O = 512
ALO3_O = 896
L2B_W = 1280


def _build_nc():
    nc = bass.Bass()
    l1_t = nc.declare_dram_parameter("l1_t", [IBLK, L1_W], BF16, isOutput=False)
    l1b_t = nc.declare_dram_parameter("l1b_t", [IBLK, L1B_W], BF16, isOutput=False)
    l2a_t = nc.declare_dram_parameter("l2a_t", [IBLK, L2A_W], BF16, isOutput=False)
    l2b_t = nc.declare_dram_parameter("l2b_t", [IBLK, L2B_W], BF16, isOutput=False)
    out = nc.declare_dram_parameter("out", [NSHARD, M], BF16, isOutput=True)

    with tile.TileContext(nc) as tc:
        with (
            tc.tile_pool(name="inp", bufs=1) as inp_pool,
            tc.tile_pool(name="stage", bufs=1) as stage_pool,
            tc.tile_pool(name="ps", bufs=1, space="PSUM") as ps_pool,
        ):
            dma_insts = []
            l1_sb = inp_pool.tile([IBLK, L1_W], BF16, tag="l1")
            dma_insts.append(nc.sync.dma_start(out=l1_sb, in_=l1_t[:, :]))
            l1b_sb = inp_pool.tile([IBLK, L1B_W], BF16, tag="l1b")
            dma_insts.append(nc.sync.dma_start(out=l1b_sb, in_=l1b_t[:, :]))
            l2a_sb = inp_pool.tile([IBLK, L2A_W], BF16, tag="l2a")
            dma_insts.append(nc.sync.dma_start(out=l2a_sb, in_=l2a_t[:, :]))

            def rows(s, k=32):
                return slice(32 * s, 32 * s + k)

            def bh(h, s):
                sb, o = (l1_sb, BH0_O) if h == 0 else (l2a_sb, BH1_O)
                return sb[rows(s), o : o + JBLK]

            def bl(h, s):
                sb, o = (l1b_sb, BL0_O) if h == 0 else (l2a_sb, BL1_O)
                return sb[rows(s), o : o + JBLK]

            def ynr(h, s):
                sb, o = (l1b_sb, YN0_O) if h == 0 else (l2b_sb, YN1_O)
                return sb[rows(s, 3), o : o + JBLK]

            xn_bias = l1_sb[:, XN_O : XN_O + 2 * NI].bitcast(mybir.dt.float32)

            # Tiny ACT-engine read of l1 so the scalar engine observes the l1
            # DMA semaphore here (1 wait); the real activations then carry
            # only their PE wait (walrus rejects multi-wait ACTIVATE, and
            # Tile doesn't track that the PE wait transitively covers l1).
            l2b_sb = inp_pool.tile([IBLK, L2B_W], BF16, tag="l2b")
            dma_insts.append(nc.sync.dma_start(out=l2b_sb, in_=l2b_t[:, :]))

            scratch = inp_pool.tile([IBLK, 1], mybir.dt.float32, tag="scr")
            nc.scalar.copy(out=scratch, in_=l1_sb[:, 0:1])

            ps_tiles = [
                ps_pool.tile(
                    [IBLK, PSW], mybir.dt.float32, tag=f"ps{h}", name=f"ps{h}"
                )
                for h in range(NH)
            ]

            out_sb = stage_pool.tile([IBLK, NI * M], BF16, tag="out", name="out")

            def store(eng, row0, nrow, col0, ncol, sb_col0):
                """out[row0:row0+nrow, col0:col0+ncol] <- out_sb[:, sb_col0..]

                nrow may span multiple 128-row blocks (nrow = 128*b); the
                matching SBUF columns are b consecutive ncol-wide groups.
                """
                b = nrow // IBLK
                d_ap = out[row0 : row0 + nrow, col0 : col0 + ncol].rearrange(
                    "(b p) j -> p b j", b=b
                )
                s_ap = out_sb[:, sb_col0 : sb_col0 + b * ncol].rearrange(
                    "p (b j) -> p b j", b=b
                )
                dma_insts.append(eng.dma_start(out=d_ap, in_=s_ap))

            act_insts = []
            mm_insts = []
            for i in range(NI):
                for h in range(NH):
                    ps = ps_tiles[h]
                    # 4 passes x 4 strips; strip s = column slice q=s of the
                    # PSUM tile. Inner loop cycles strips so consecutive
                    # matmuls run in different row groups (concurrent).
                    if i == 0:
                        ahi_sb, ahi_o = l1_sb, AHI_O
                        alo_sb, alo_o = l1_sb, ALO_O
                    else:
                        ahi_sb, ahi_o = l2b_sb, AHI3_O + (i - 1) * IBLK
                        alo_sb, alo_o = l2b_sb, ALO3_O + (i - 1) * IBLK
                    for p in range(4):
                        start = p == 0
                        stop = p == 3
                        for s in range(NSTRIP):
                            if p == 0:
                                lhsT = ahi_sb[rows(s), ahi_o : ahi_o + IBLK]
                                rhs = bh(h, s)
                            elif p == 1:
                                lhsT = alo_sb[rows(s), alo_o : alo_o + IBLK]
                                rhs = bh(h, s)
                            elif p == 2:
                                lhsT = ahi_sb[rows(s), ahi_o : ahi_o + IBLK]
                                rhs = bl(h, s)
                            else:
                                lhsT = l1b_sb[rows(s, 3), ONES_O : ONES_O + IBLK]
                                rhs = ynr(h, s)
                            mm_insts.append(
                                nc.tensor.matmul(
                                    ps[:, s * JBLK : (s + 1) * JBLK],
                                    lhsT=lhsT,
                                    rhs=rhs,
                                    start=start,
                                    stop=stop,
                                    tile_position=(32 * s, 0),
                                )
                            )
                    if i == NI - 1 and h == NH - 1:
                        # Final tile: two 1024-wide ACTs so the last store
                        # is only 256KB (the tail after the chain is
                        # trigger+pipe+transfer of the LAST chunk).
                        HP = PSW // 2
                        for q in range(2):
                            act_insts.append(
                                nc.scalar.activation(
                                    out=out_sb[
                                        :,
                                        i * M + h * PSW + q * HP : i * M
                                        + h * PSW
                                        + (q + 1) * HP,
                                    ],
                                    in_=ps[:, q * HP : (q + 1) * HP],
                                    func=mybir.ActivationFunctionType.Exp,
                                    bias=xn_bias[:, i : i + 1],
                                    scale=1.0,
                                )
                            )
                            store(
                                nc.scalar, 3 * IBLK, IBLK,
                                h * PSW + q * HP, HP,
                                3 * M + h * PSW + q * HP,
                            )
                    else:
                        act_insts.append(
                            nc.scalar.activation(
                                out=out_sb[
                                    :, i * M + h * PSW : i * M + (h + 1) * PSW
                                ],
                                in_=ps,
                                func=mybir.ActivationFunctionType.Exp,
                                bias=xn_bias[:, i : i + 1],
                                scale=1.0,
                            )
                        )
                    # Store schedule (9 DMAs total incl. 4 loads; the 9th
                    # reuses DMAHW lane 0 — safe because per-queue HWDGE
                    # completions are FIFO and lane-sem waits are cumulative;
                    # the Tile-added lane-ordering dep is demoted below):
                    #   after (i,h1), i<3: [i] full 1MB     -> sync
                    #   after (i3,h0): [i3 h0] 512KB        -> sync
                    #   after (i3,h1): two 256KB halves     -> sync + scalar
                    # (the scalar trigger sits after the last ACT, so it
                    # cannot bubble the chain)
                    if i < 3 and h == 1:
                        store(nc.sync, i * IBLK, IBLK, 0, M, i * M)
                    elif i == 3 and h == 0:
                        store(nc.sync, 3 * IBLK, IBLK, 0, PSW, 3 * M)


            # Demote ACT->ACT pseudo-deps (PSUM bank read-read serialization,
            # already ordered through the interleaved matmuls + same-engine
            # FIFO) to nosync: walrus rejects multi-wait ACTIVATE.
            import bass_rust as _br  # noqa: PLC0415

            act_names = {a.ins.name for a in act_insts}
            for a in act_insts:
                deps = list(a.ins.sync_dependency_names())
                spurious = [d for d in deps if d in act_names]
                if spurious:
                    keep = [d for d in deps if d not in act_names]
                    a.ins.take_sync_dependencies()
                    a.ins.set_sync_dependencies(
                        _br.InstructionNameOrderedSet(keep)
                    )
                    a.ins.add_nosync_dependencies_from(
                        _br.InstructionNameOrderedSet(spurious)
                    )

            # Stores of the shared stage tile depend on every ACT that wrote
            # their column range; all ACTs are scalar-FIFO ordered, so only
            # the LAST producer's sem wait is needed (walrus single-wait).
            for dins in dma_insts[4:]:
                on_scalar = dins.ins.engine == mybir.EngineType.Activation
                deps = list(dins.ins.sync_dependency_names())
                adeps = [d for d in deps if d in act_names]
                if adeps and (len(adeps) > 1 or on_scalar):
                    last = max(adeps, key=lambda d: int(d.split("-")[1]))
                    if on_scalar:
                        keep = [d for d in deps if d not in act_names]
                        spurious = adeps
                    else:
                        keep = [
                            d for d in deps if d not in act_names or d == last
                        ]
                        spurious = [d for d in adeps if d != last]
                    dins.ins.take_sync_dependencies()
                    dins.ins.set_sync_dependencies(
                        _br.InstructionNameOrderedSet(keep)
                    )
                    dins.ins.add_nosync_dependencies_from(
                        _br.InstructionNameOrderedSet(spurious)
                    )

            # Demote DMA->DMA lane-reuse ordering deps to nosync: HWDGE
            # completions are FIFO per queue and every lane-sem wait uses a
            # cumulative value, so cross-DMA order never affects a consumer.
            dma_names = {dd.ins.name for dd in dma_insts}
            for dins in dma_insts:
                deps = list(dins.ins.sync_dependency_names())
                spurious = [d for d in deps if d in dma_names]
                if spurious:
                    keep = [d for d in deps if d not in dma_names]
                    dins.ins.take_sync_dependencies()
                    dins.ins.set_sync_dependencies(
                        _br.InstructionNameOrderedSet(keep)
                    )
                    dins.ins.add_nosync_dependencies_from(
                        _br.InstructionNameOrderedSet(spurious)
                    )

            # Wait-funnel so the framework's kernel-tail drain needs no waits
            # of its own (walrus rejects its usual all-sems wait list). Each
            # nop also nosync-orders after every DMA so the scheduler cannot
            # slot a slow-waiting nop ahead of a still-pending store on the
            # same queue (head-of-line blocking).
            for t in [mm_insts[-1], act_insts[-1], *dma_insts]:
                nop = nc.sync.nop(nofuse=True, hint="tail_funnel")
                add_dep_helper(nop.ins, t.ins, True, "tail wait funnel")
                for dd in dma_insts:
                    if dd is not t:
                        add_dep_helper(nop.ins, dd.ins, False, "funnel order")
    return nc


def _bf16_splits(x, n):
    """Split fp32 array into n bf16 parts summing to ~x."""
    parts = []
    rem = x.astype(np.float32)
    for _ in range(n):
        p = rem.astype(NPBF16)
        parts.append(p)
        rem = rem - p.astype(np.float32)
    return parts


def run(x1, x2, log_rho, log_sigma, trace=False):
    """Returns (K, exec_time_ns). exec_time_ns is None unless trace=True."""
    x1 = np.asarray(x1, dtype=np.float32)
    x2 = np.asarray(x2, dtype=np.float32)
    rho = float(np.exp(np.float64(np.asarray(log_rho))))
    log_sig = 2.0 * float(np.asarray(log_sigma))  # log(sigma)

    xs = (x1 / np.float32(rho)).astype(np.float32)
    ys = (x2 / np.float32(rho)).astype(np.float32)
    xn = np.einsum("np,np->n", xs, xs, dtype=np.float64)
    yn = np.einsum("mp,mp->m", ys, ys, dtype=np.float64)

    a = xs.T.astype(np.float32)  # (32, N)
    b = ys.T.astype(np.float32)  # (32, M)
    a_hi, a_lo = _bf16_splits(a, 2)
    b_hi, b_lo = _bf16_splits(b, 2)
    y1, y2, y3 = _bf16_splits((-0.5 * yn).astype(np.float32), 3)
    # per-row ACT bias: -0.5*||x_i||^2 + log(sigma), exact fp32
    xbias = ((-0.5 * xn) + log_sig).astype(np.float32)

    def pack_b(src, h):
        # strip s rows hold the columns strip s consumes: B[:, h*PSW+s*JBLK..]
        o = np.zeros((IBLK, JBLK), NPBF16)
        for s in range(NSTRIP):
            o[32 * s : 32 * s + 32] = src[:, h * PSW + s * JBLK : h * PSW + (s + 1) * JBLK]
        return o

    def pack_yn(h):
        o = np.zeros((IBLK, JBLK), NPBF16)
        for s in range(NSTRIP):
            for r, yr in enumerate((y1, y2, y3)):
                o[32 * s + r] = yr[h * PSW + s * JBLK : h * PSW + (s + 1) * JBLK]
        return o

    ones = np.zeros((IBLK, IBLK), NPBF16)
    for s in range(NSTRIP):
        ones[32 * s : 32 * s + 3] = NPBF16(1.0)

    l1b = np.zeros((IBLK, L1B_W), NPBF16)
    l1b[:, BL0_O : BL0_O + JBLK] = pack_b(b_lo, 0)
    l1b[:, YN0_O : YN0_O + JBLK] = pack_yn(0)
    l1b[:, ONES_O : ONES_O + IBLK] = ones

    l2a = np.zeros((IBLK, L2A_W), NPBF16)
    l2a[:, BH1_O : BH1_O + JBLK] = pack_b(b_hi, 1)
    l2a[:, BL1_O : BL1_O + JBLK] = pack_b(b_lo, 1)

    nc = _build_nc()
    in_maps = []
    for c in range(NCORES):
        s0 = slice(c * NSHARD, c * NSHARD + IBLK)
        s123 = slice(c * NSHARD + IBLK, (c + 1) * NSHARD)
        l1 = np.zeros((IBLK, L1_W), NPBF16)
        l2b = np.zeros((IBLK, L2B_W), NPBF16)
        l2b[:, YN1_O : YN1_O + JBLK] = pack_yn(1)
        for s in range(NSTRIP):
            r = slice(32 * s, 32 * s + 32)
            l1[r, AHI_O : AHI_O + IBLK] = a_hi[:, s0]
            l1[r, ALO_O : ALO_O + IBLK] = a_lo[:, s0]
            l2b[r, AHI3_O : AHI3_O + 3 * IBLK] = a_hi[:, s123]
            l2b[r, ALO3_O : ALO3_O + 3 * IBLK] = a_lo[:, s123]
        l1[:, BH0_O : BH0_O + JBLK] = pack_b(b_hi, 0)
        # fp32 bias bits ride along as bf16 column pairs
        xb = np.zeros((IBLK, NI), np.float32)
        for i in range(NI):
            xb[:, i] = xbias[c * NSHARD + i * IBLK : c * NSHARD + (i + 1) * IBLK]
        l1[:, XN_O : XN_O + 2 * NI] = xb.view(np.uint16).view(NPBF16)
        in_maps.append(
            {
                "l1_t": np.ascontiguousarray(l1),
                "l1b_t": l1b,
                "l2a_t": l2a,
                "l2b_t": np.ascontiguousarray(l2b),
            }
        )

    res = run_bass_kernel_spmd(
        nc, in_maps, core_ids=list(range(NCORES)), trace=trace
    )
    full = np.concatenate(
        [res.results[c]["out"].astype(np.float32) for c in range(NCORES)],
        axis=0,
    )
    return full, res.exec_time_ns


def kernel(x1, x2, log_rho, log_sigma):
    out, _ = run(x1, x2, log_rho, log_sigma, trace=False)
    return out
